# revision 56
# baseline (speedup 1.0000x reference)
"""Trainium2 Bass kernel for nn_DiscretePolicy (gnn_message_passing).

Reference computation:
  Xn = batchnorm(X)  (training-mode, biased var, eps=1e-5)
  ent = Xn[:, 4:].reshape(B, 100, 2)
  me = leaky_relu(ent @ W_me.T + b_me); me_out = mean_k(me)      # [B, 64]
  h = leaky_relu([Xn[:, :4], me_out] @ W1.T + b1)
  h = leaky_relu(h @ W2.T + b2)
  out = softmax(h @ W3.T + b3)

Strategy (8-way batch-parallel, 2048 rows/core):
  - X is pre-expanded on the host to a bf16 column-layout XB [B, 256]
    (block k of 32 cols <- features 28k..28k+31; pad features zero), and
    rotated per-core so chunk 0 is the core's own shard.
  - BatchNorm stats: every core streams the FULL bf16 batch and reduces
    locally (a cross-core AllReduce costs ~135us cold in this harness).
    Sums via PE ones-matmuls on 4 concurrent col-group accumulation
    chains (x-even/x-odd/sq-even/sq-odd); squares on DVE (2x bf16).
  - Own-shard tiles are PE-transposed to feature-major XT [128, 2*2048],
    normalized in place with the batch stats (rsqrt via reciprocal+sqrt).
  - leaky_relu(z) is decomposed as alpha*z + (1-alpha)*relu(z). The linear
    part is folded analytically into the first MLP layer; only
    R = sum_k relu(z_k + b_me) is computed at full resolution:
      * entity matmuls: K=32 zero-padded block weights, one entity-pair per
        matmul, 4 concurrent via tile_position row groups, bf16 PSUM
      * relu+bias split 3 ways across DVE (2x bf16), ACT, and GpSimd
      * pooling over entities: two concurrent PE accumulation chains on
        col-group halves (tile_position=(0,0)/(0,64)), bf16 sel weights
  - MLP in bf16: leaky layers via max identity — h = a*p + (1-a)*relu(p)
    as two accumulating matmuls on (p, relu(p)); softmax via PE transpose
    to batch-major then Exp + reciprocal (logits are O(1): no max-sub).
"""

import sys
import numpy as np

sys.path.insert(0, "/opt/trn_rl_repo")

import ml_dtypes

B_FULL, D, H, A = 16384, 204, 64, 32
NCORES = 8
BL = B_FULL // NCORES          # 2048 rows per core
NBT = 4                        # batch tiles per core
NT = BL // NBT                 # 512 columns per batch tile
K_ENT = 100                    # entities
NPAIR = 50                     # entity pairs (2 entities / matmul)
ALPHA = 0.01                   # jax.nn.leaky_relu default negative_slope
EPS = 1e-5
C = 256                        # padded feature columns (bf16 layout)
DPAD = 228                     # padded feature count (204..227 = 0)

# --- column layout: block k of 32 sbuf columns <- features 28k .. 28k+31 ---
# Pair p (features 4+4p..7+4p) lives in block k=(4+4p)//28 at column
# 4+4p+4k; pairs are 4-aligned and blocks start at multiples of 28 (also
# 4-aligned), so pairs never straddle blocks.
PAIR_COL = np.array([4 + 4 * p + 4 * ((4 + 4 * p) // 28) for p in range(NPAIR)])
for p in range(NPAIR):
    c = PAIR_COL[p]
    k = c // 32
    assert c % 4 == 0 and c % 32 <= 24 and 28 * k <= 4 + 4 * p <= 28 * k + 24


def _feat_of_col():
    f = np.full(C, -1, np.int64)
    for c in range(C):
        k, r = c // 32, c % 32
        if 28 * k + r < D:
            f[c] = 28 * k + r
    return f


FEAT_OF_COL = _feat_of_col()

PAIR_FILL = PAIR_COL // 128            # which transpose block (XT region)
PAIR_PART = PAIR_COL % 128             # partition of first row
PAIR_QUAD = PAIR_PART // 32            # row-group quadrant
PAIR_SLOT = (PAIR_PART % 32) // 4      # slot within quadrant (selects lhsT block)

# round-robin issue order across quadrants
_QLISTS = [[p for p in range(NPAIR) if PAIR_QUAD[p] == g] for g in range(4)]
PAIR_ORDER = []
for t in range(max(len(q) for q in _QLISTS)):
    for g in range(4):
        if t < len(_QLISTS[g]):
            PAIR_ORDER.append(_QLISTS[g][t])
assert len(PAIR_ORDER) == NPAIR

_prog_cache = {}


def _build_host_constants(W_me, b_me, W1, b1, W2, b2, W3, b3):
    bf16 = ml_dtypes.bfloat16
    # Wall [128, 8*128]: for quadrant row r (0..31) and slot m: rows 4m..4m+3
    # hold the entity-pair weight block, other rows zero.  Wall same for all
    # quadrants -> replicate pattern to all 128 partitions.
    pat = np.zeros((32, 8 * 128), np.float32)
    for m in range(8):
        for j in range(2):          # entity within pair
            for e in range(2):      # input dim
                # row 4m+2j+e, columns m*128 + (64j .. 64j+63) = W_me[:, e]
                pat[4 * m + 2 * j + e, m * 128 + 64 * j: m * 128 + 64 * (j + 1)] = W_me[:, e]
        # row 28 (a never-used duplicate column in every quadrant) is set
        # to 1.0 in xt after normalization; putting b_me here folds the
        # bias add into the z matmul, so relu is a single-op max.
        pat[28, m * 128:(m + 1) * 128] = np.tile(b_me, 2)
    Wall = np.tile(pat, (4, 1)).astype(bf16)

    sel = np.zeros((128, 64), np.float32)
    for j in range(2):
        sel[np.arange(64) + 64 * j, np.arange(64)] = 1.0
    selpack = np.concatenate([sel, sel], axis=1).astype(ml_dtypes.float8_e4m3)

    # m-vector masks (feature sums of the pair columns), folded below into
    # the h1 lhsT weights together with the head columns.
    mA2 = np.zeros((128, 2), np.float32)
    mB2 = np.zeros((128, 2), np.float32)
    pair_cols = set()
    for p in range(NPAIR):
        for off in range(4):
            pair_cols.add(int(PAIR_COL[p]) + off)
    for c in range(C):
        f = FEAT_OF_COL[c]
        if c in pair_cols and f >= 4:
            (mA2 if c < 128 else mB2)[c % 128, (f - 4) % 2] = 1.0

    # h1 = (1-a)/K * W1b @ R  +  a/K * (W1b@W_me) @ m_raw  +  W1h @ head:
    # R arrives as two evacuated pool halves (lhsT_R applied to each);
    # the m_raw and head contributions are folded into direct lhsT weights
    # applied to the normalized xt regions.
    W1h = W1[:, :4]
    W1b = W1[:, 4:]
    lhsT_R = (((1.0 - ALPHA) / K_ENT) * W1b.T).astype(bf16).copy()
    coefT = ((ALPHA / K_ENT) * (W1b @ W_me)).T          # [2, 64]
    lhsT_mA = mA2 @ coefT
    lhsT_mA[0:4, :] += W1h.T
    lhsT_mA = lhsT_mA.astype(bf16)
    lhsT_mB = (mB2 @ coefT).astype(bf16)
    b1vec = (b1 + ALPHA * (W1b @ b_me)).reshape(64, 1).astype(np.float32)

    # stacked leaky weights: h_next = lhsT[[a*W],[ (1-a)*W ]] @ [p; relu(p)]
    lhsT_h2 = np.concatenate([(ALPHA * W2).T, ((1.0 - ALPHA) * W2).T],
                             axis=0).astype(bf16)        # [128, 64]
    b2vec = b2.reshape(64, 1).astype(np.float32)
    lhsT_h3 = np.concatenate([(ALPHA * W3).T, ((1.0 - ALPHA) * W3).T],
                             axis=0).astype(bf16)        # [128, 32]
    b3vec = b3.reshape(32, 1).astype(np.float32)

    ident = np.eye(128, dtype=np.float32).astype(bf16)   # logits transpose identity
    ident32 = np.eye(32, dtype=np.float32).astype(bf16)  # logits transpose identity
    onesb = np.ones((128, 32), np.float32).astype(bf16)  # stats ones (M=32: HAM-visible)

    return dict(Wall=Wall, selpack=selpack, lhsT_R=lhsT_R, lhsT_mA=lhsT_mA,
                lhsT_mB=lhsT_mB, b1vec=b1vec, lhsT_h2=lhsT_h2, b2vec=b2vec,
                lhsT_h3=lhsT_h3, b3vec=b3vec, ident=ident, ident32=ident32,
                onesb=onesb)


_COL_MAP = np.array([28 * (c // 32) + c % 32 for c in range(C)])


def _expand_X(X):
    """Host-side: pad, expand to the 256-col window layout, cast bf16."""
    Xp = np.zeros((X.shape[0], DPAD), np.float32)
    Xp[:, :D] = X
    return np.ascontiguousarray(Xp[:, _COL_MAP].astype(ml_dtypes.bfloat16))


def make_in_maps(inputs):
    X = np.asarray(inputs["X"], np.float32)
    consts = _build_host_constants(
        np.asarray(inputs["W_me"], np.float32), np.asarray(inputs["b_me"], np.float32),
        np.asarray(inputs["W1"], np.float32), np.asarray(inputs["b1"], np.float32),
        np.asarray(inputs["W2"], np.float32), np.asarray(inputs["b2"], np.float32),
        np.asarray(inputs["W3"], np.float32), np.asarray(inputs["b3"], np.float32),
    )
    XB = _expand_X(X)
    in_maps = []
    for i in range(NCORES):
        rot = np.ascontiguousarray(np.concatenate([XB[i * BL:], XB[:i * BL]], axis=0))
        m = {"XB": rot}
        m.update(consts)
        in_maps.append(m)
    return in_maps


def build_program(num_devices=NCORES):
    """Emit the SPMD Bass program (identical on every core)."""
    from contextlib import ExitStack
    import concourse.bass as bass
    import concourse.bacc as bacc
    import concourse.tile as tile
    from concourse import mybir

    fp32 = mybir.dt.float32
    bf16 = mybir.dt.bfloat16
    fp8 = mybir.dt.float8e4
    ALU = mybir.AluOpType
    ACTF = mybir.ActivationFunctionType

    nc = bacc.Bacc(None, num_devices=num_devices)

    XB = nc.declare_dram_parameter("XB", [B_FULL, C], bf16, isOutput=False)
    OUT = nc.declare_dram_parameter("OUT", [BL, A], fp32, isOutput=True)
    dparams = {}
    for name, shape, dt in [
        ("Wall", [128, 1024], bf16), ("selpack", [128, 128], fp8),
        ("lhsT_R", [64, 64], bf16), ("lhsT_mA", [128, 64], bf16),
        ("lhsT_mB", [128, 64], bf16),
        ("b1vec", [64, 1], fp32), ("lhsT_h2", [128, 64], bf16),
        ("b2vec", [64, 1], fp32), ("lhsT_h3", [128, 32], bf16),
        ("b3vec", [32, 1], fp32), ("ident", [128, 128], bf16),
        ("ident32", [32, 32], bf16), ("onesb", [128, 32], bf16),
    ]:
        dparams[name] = nc.declare_dram_parameter(name, shape, dt, isOutput=False)

    ntile = BL // 128              # 16 tile-rows per chunk
    nchunk = B_FULL // BL          # 8 chunks (chunk 0 = own shard)

    with tile.TileContext(nc) as tc, ExitStack() as ctx:
        singles = ctx.enter_context(tc.tile_pool(name="singles", bufs=1))
        xtp = ctx.enter_context(tc.tile_pool(name="xtp", bufs=1))
        x0p = ctx.enter_context(tc.tile_pool(name="x0p", bufs=1))
        xstream = ctx.enter_context(tc.tile_pool(name="xstream", bufs=2))

        cst = {}

        def load_const(name):
            p = dparams[name]
            t = singles.tile(list(p.shape), p.dtype, tag=f"cst_{name}")
            nc.sync.dma_start(out=t[:], in_=p[:])
            cst[name] = t

        load_const("onesb")
        load_const("ident")

        # PE heater: the HAM clock gate keeps the PE at 1.2 GHz unless its
        # activity window reads busy.  Dummy wide matmuls (ident x ident)
        # fill idle gaps so the whole kernel runs at 2.4 GHz.
        heatp = ctx.enter_context(tc.tile_pool(name="heatp", bufs=1, space="PSUM"))
        heat = heatp.tile([128, 512], fp32, tag="heat")

        def heater(n=1):
            for _ in range(n):
                nc.tensor.matmul(heat[:, 0:128], cst["ident"][:], cst["ident"][:],
                                 start=True, stop=True)

        def heater512(src):
            # big heater: 512-free dummy matmul reading live SBUF data
            nc.tensor.matmul(heat[:], cst["ident"][:], src, start=True, stop=True)

        heater(30)   # ~3.5us of back-to-back matmuls: un-throttle before stats

        def chunk_dma(ci, pool):
            xc = pool.tile([128, ntile, C], bf16)
            cin = bass.AP(
                tensor=XB[:].tensor, offset=XB[:].offset + ci * BL * C,
                ap=[[ntile * C, 128], [C, ntile], [1, C]],
            )
            nc.sync.dma_start(out=xc[:], in_=cin)
            return xc

        xc0 = chunk_dma(0, x0p)         # own shard: stays resident
        xchunks = {1: chunk_dma(1, xstream)}

        for name in dparams:
            if name not in cst:
                load_const(name)

        # XT: feature-major bf16, region F at cols F*BL .. F*BL+BL
        xt = xtp.tile([128, 2 * BL], bf16)
        xt3 = xt.rearrange("p (s n) -> p s n", s=2)

        # ---------------- pre-phase: load, stats, transpose ----------------
        with ExitStack() as pre:
            sps = pre.enter_context(tc.tile_pool(name="sps", bufs=1, space="PSUM"))
            stp = pre.enter_context(tc.tile_pool(name="stp", bufs=4))
            xsqp = pre.enter_context(tc.tile_pool(name="xsqp", bufs=2))

            # stats: 4 concurrent col-group accumulation chains (x even
            # tiles, x odd tiles, sq even, sq odd) with M=32 outputs so the
            # matmuls register as HAM activity (M=1 sums are invisible to
            # the clock-gate monitor and the PE stays at half clock)
            pstiles = []
            for ch in range(4):
                ps_stc = sps.tile([128, C], fp32, tag=f"ps_st{ch}", name=f"ps_st{ch}")
                pstiles.append(ps_stc)
            onesb = cst["onesb"][:]
            first = [True, True, True, True]

            def stats_mms(xc, ci):
                xq = xsqp.tile([128, ntile, C], bf16)
                nc.vector.tensor_tensor(
                    out=xq.rearrange("p t d -> p (t d)")[:],
                    in0=xc.rearrange("p t d -> p (t d)")[:],
                    in1=xc.rearrange("p t d -> p (t d)")[:], op=ALU.mult)
                last = ci == nchunk - 1
                for i in range(ntile):
                    ch = i % 2
                    nc.tensor.matmul(pstiles[ch][ch * 32:ch * 32 + 32, :], onesb,
                                     xc[:, i, :],
                                     start=first[ch], stop=(last and i >= ntile - 2),
                                     tile_position=(0, 32 * ch))
                    first[ch] = False
                    chq = 2 + ch
                    nc.tensor.matmul(pstiles[chq][chq * 32:chq * 32 + 32, :], onesb,
                                     xq[:, i, :],
                                     start=first[chq], stop=(last and i >= ntile - 2),
                                     tile_position=(0, 32 * chq))
                    first[chq] = False

            stats_mms(xc0, 0)
            # transposes of the own shard via the DMA-engine transpose XBAR
            # (zero PE cost; overlaps the rest of the stats stream)
            for i in range(ntile):
                nc.sync.dma_start(out=xt[:, 128 * i:128 * (i + 1)],
                                  in_=xc0[:, i, 0:128], transpose=True)
                nc.sync.dma_start(out=xt[:, BL + 128 * i:BL + 128 * (i + 1)],
                                  in_=xc0[:, i, 128:256], transpose=True)
            for ci in range(1, nchunk):
                xc = xchunks.pop(ci) if ci in xchunks else chunk_dma(ci, xstream)
                stats_mms(xc, ci)

            # --- stats: evacuate (fold 4 chains -> 2), reshape, normalize ---
            # (only one PSUM operand allowed per instruction: stage the odd
            # chains through SBUF first)
            st_odd = stp.tile([1, 2 * C], fp32, tag="st_odd")
            nc.scalar.activation(st_odd[:, 0:C], pstiles[1][32:33, :], ACTF.Copy)
            nc.scalar.activation(st_odd[:, C:2 * C], pstiles[3][96:97, :], ACTF.Copy)
            st_sb = stp.tile([1, 2 * C], fp32)
            nc.vector.tensor_tensor(out=st_sb[:, 0:C], in0=pstiles[0][0:1, :],
                                    in1=st_odd[:, 0:C], op=ALU.add)
            nc.vector.tensor_tensor(out=st_sb[:, C:2 * C], in0=pstiles[2][64:65, :],
                                    in1=st_odd[:, C:2 * C], op=ALU.add)
            # poison the stats of the 8 never-used duplicate columns
            # (row 28 of each quadrant, both regions) so normalization
            # writes exactly 1.0 there: with sum=-B*sqrt(M), sumsq=2*B*M,
            # mu=-sqrt(M), var=M -> s=M^-0.5=1e-6, shift=-mu*s=1.0.
            BIGM = 1e12
            for base in (28, 28 + 128):
                sx = bass.AP(tensor=st_sb.tensor, offset=st_sb.offset + base,
                             ap=[[st_sb.ap[0][0], 1], [32, 4]])
                sq = bass.AP(tensor=st_sb.tensor,
                             offset=st_sb.offset + C + base,
                             ap=[[st_sb.ap[0][0], 1], [32, 4]])
                nc.vector.memset(sx, -B_FULL * BIGM ** 0.5)
                nc.vector.memset(sq, 2.0 * B_FULL * BIGM)
            cc_out = nc.dram_tensor("cc_out", [1, 2 * C], fp32)
            nc.sync.dma_start(out=cc_out[:], in_=st_sb[:])
            cc_v = cc_out[:].rearrange("a (e p) -> (a p) e", e=2)
            # per-partition stats for both regions at once: st[p, F] layout
            st = stp.tile([128, 2, 2], fp32, tag="st")   # [p, F, (sx, sq)]
            for F in range(2):
                nc.sync.dma_start(out=st[:, F, :], in_=cc_v[128 * F:128 * (F + 1), :])
            stf = st.rearrange("p f e -> p (f e)")
            muex = stp.tile([128, 4], fp32, tag="muex")  # (muA, ex2A, muB, ex2B)
            nc.vector.tensor_scalar(out=muex[:], in0=stf[:, 0:4], scalar1=1.0 / B_FULL,
                                    scalar2=None, op0=ALU.mult)
            mus = muex[:, 0:4:2]
            ex2 = muex[:, 1:4:2]
            mu2 = stp.tile([128, 2], fp32, tag="mu2")
            nc.vector.tensor_tensor(out=mu2[:], in0=mus, in1=mus, op=ALU.mult)
            vpe = stp.tile([128, 2], fp32, tag="vpe")
            # vpe = (mu2 * -1 + ex2) + eps
            nc.vector.scalar_tensor_tensor(out=vpe[:], in0=mu2[:], scalar=-1.0,
                                           in1=ex2, op0=ALU.mult, op1=ALU.add)
            nc.vector.tensor_scalar(out=vpe[:], in0=vpe[:], scalar1=EPS, scalar2=None,
                                    op0=ALU.add)
            heater(14)   # cover the stats-finalize + normalize PE gap
            rs = stp.tile([128, 2], fp32, tag="rs")
            nc.vector.reciprocal(rs[:], vpe[:])          # 1/(var+eps)
            svec = stp.tile([128, 2], fp32, tag="sv")
            nc.scalar.activation(svec[:], rs[:], ACTF.Sqrt)
            nmvec = stp.tile([128, 2], fp32, tag="nm")
            # nm = (mu * -1) * s
            nc.vector.scalar_tensor_tensor(out=nmvec[:], in0=mus, scalar=-1.0,
                                           in1=svec[:], op0=ALU.mult, op1=ALU.mult)
            for F in range(2):
                nc.vector.tensor_scalar(
                    out=xt[:, BL * F:BL * (F + 1)], in0=xt[:, BL * F:BL * (F + 1)],
                    scalar1=svec[:, F:F + 1], scalar2=nmvec[:, F:F + 1],
                    op0=ALU.mult, op1=ALU.add)

        # ---------------- main phase ----------------
        zpsp = ctx.enter_context(tc.tile_pool(name="zpsp", bufs=1, space="PSUM"))
        accp = ctx.enter_context(tc.tile_pool(name="accp", bufs=1, space="PSUM"))
        mlpp = ctx.enter_context(tc.tile_pool(name="mlpp", bufs=1, space="PSUM"))
        ypool = ctx.enter_context(tc.tile_pool(name="ypool", bufs=4))
        ypoolb = ctx.enter_context(tc.tile_pool(name="ypoolb", bufs=4))
        mlps = ctx.enter_context(tc.tile_pool(name="mlps", bufs=4))
        outp = ctx.enter_context(tc.tile_pool(name="outp", bufs=4))

        zpsA = zpsp.tile([128, 2 * 512], fp32, tag="zpsA")
        zpsB = zpsp.tile([128, 2 * 512], fp32, tag="zpsB")

        groups = [PAIR_ORDER[i:i + 2] for i in range(0, NPAIR, 2)]
        ngrp = len(groups)           # 25 groups of 2 pairs
        DVE_RELU = {0, 2, 4, 7, 9, 11, 13, 16, 18, 20, 22, 24}  # 12 DVE / 13 ACT

        for bt in range(NBT):
            col0 = bt * NT
            # two pool accumulation chains in separate PSUM banks (the
            # col-group halves may not share a bank's accumulation group)
            acc0 = accp.tile([128, NT], fp32, tag="acc0")
            acc1 = accp.tile([128, NT], fp32, tag="acc1")

            def emit_z(gi):
                zps = zpsA if gi % 2 == 0 else zpsB
                for j, p in enumerate(groups[gi]):
                    g = PAIR_QUAD[p]
                    m = PAIR_SLOT[p]
                    F = PAIR_FILL[p]
                    nc.tensor.matmul(
                        zps[:, j * 512:(j + 1) * 512],
                        cst["Wall"][32 * g:32 * (g + 1), 128 * m:128 * (m + 1)],
                        xt[32 * g:32 * (g + 1), BL * F + col0:BL * F + col0 + NT],
                        start=True, stop=True,
                        tile_position=(32 * int(g), 0),
                    )

            ytiles = {}
            selpack3 = cst["selpack"][:].rearrange("p (two f) -> p two f", two=2)

            def emit_relu(gi):
                zps = zpsA if gi % 2 == 0 else zpsB
                y = (ypool if gi % 2 == 0 else ypoolb).tile([128, 2 * 512], fp8)
                ytiles[gi] = y
                zv = zps[:, 0:2 * 512]
                # bias already folded into the z matmul: single-op relu
                # (GpSimd cannot read PSUM, so only DVE and ACT share it)
                if gi in DVE_RELU:
                    nc.vector.tensor_scalar(out=y[:], in0=zv, scalar1=0.0,
                                            scalar2=None, op0=ALU.max)
                else:
                    nc.scalar.activation(y[:], zv, ACTF.Relu)

            # pool chains: chain0 = DoubleRow on col groups 0-1 (DR requires
            # dst partition 0), chain1 = two plain fp8 matmuls on col groups
            # 2-3; ~17/8 group split balances the two chains' cycle counts.
            ch1 = [1, 3, 5, 8, 10, 12, 14, 16, 19, 21, 23]
            ch1set = set(ch1)
            ch0 = [gi for gi in range(ngrp) if gi not in ch1set]

            def emit_pool(gi):
                y = ytiles.pop(gi)
                if gi in ch1set:
                    nc.tensor.matmul(acc1[64:128, :], selpack3[:, 0, :],
                                     y[:, 0:512],
                                     start=(gi == ch1[0]), stop=False,
                                     tile_position=(0, 64))
                    nc.tensor.matmul(acc1[64:128, :], selpack3[:, 0, :],
                                     y[:, 512:1024],
                                     start=False, stop=(gi == ch1[-1]),
                                     tile_position=(0, 64))
                else:
                    y3 = y.rearrange("p (j d) -> p j d", j=2)
                    nc.tensor.matmul(acc0[0:64, :], selpack3, y3[:, :, :],
                                     start=(gi == ch0[0]), stop=(gi == ch0[-1]),
                                     tile_position=(0, 0),
                                     perf_mode=mybir.MatmulPerfMode.DoubleRow)

            # software pipeline: z-matmuls 2 groups ahead of relu (2 PSUM
            # banks; relu(gi-2) must precede z(gi) in program order since
            # they share zps slices), pool lags 2 more.
            for gi in range(ngrp + 4):
                if gi >= 4:
                    emit_pool(gi - 4)
                if 2 <= gi < ngrp + 2:
                    emit_relu(gi - 2)
                if gi < ngrp:
                    emit_z(gi)
                heater512(xt[:, col0:col0 + NT])

            # ---- MLP tail ----
            # evacuate the two pool halves; h1 accumulates their R
            # contributions plus the m_raw/head terms taken directly off xt
            e0 = mlps.tile([64, NT], bf16, tag="e0")
            e1 = mlps.tile([64, NT], bf16, tag="e1")
            nc.vector.tensor_copy(e0[:], acc0[0:64, :])
            nc.scalar.activation(e1[:], acc1[64:128, :], ACTF.Copy)
            ps_h1 = mlpp.tile([64, NT], fp32, tag="mlp")
            nc.tensor.matmul(ps_h1[:], cst["lhsT_R"][:], e0[:],
                             start=True, stop=False)
            nc.tensor.matmul(ps_h1[:], cst["lhsT_R"][:], e1[:],
                             start=False, stop=False)
            nc.tensor.matmul(ps_h1[:], cst["lhsT_mA"][:], xt[:, col0:col0 + NT],
                             start=False, stop=False)
            nc.tensor.matmul(ps_h1[:], cst["lhsT_mB"][:],
                             xt[:, BL + col0:BL + col0 + NT],
                             start=False, stop=True)

            # stacked [p; relu(p)] tiles: one matmul per leaky layer
            pr1 = mlps.tile([128, NT], bf16, tag="pr1")
            nc.vector.tensor_scalar(out=pr1[0:64, :], in0=ps_h1[:],
                                    scalar1=cst["b1vec"][:], scalar2=None,
                                    op0=ALU.add)
            nc.scalar.activation(pr1[64:128, :], ps_h1[:], ACTF.Relu,
                                 bias=cst["b1vec"][:])
            heater()
            ps_h2 = mlpp.tile([64, NT], fp32, tag="mlp")
            nc.tensor.matmul(ps_h2[:], cst["lhsT_h2"][:], pr1[:],
                             start=True, stop=True)
            pr2 = mlps.tile([128, NT], bf16, tag="pr2")
            nc.vector.tensor_scalar(out=pr2[0:64, :], in0=ps_h2[:],
                                    scalar1=cst["b2vec"][:], scalar2=None,
                                    op0=ALU.add)
            nc.scalar.activation(pr2[64:128, :], ps_h2[:], ACTF.Relu,
                                 bias=cst["b2vec"][:])
            heater()
            ps_lg = mlpp.tile([32, NT], fp32, tag="mlp")
            nc.tensor.matmul(ps_lg[:], cst["lhsT_h3"][:], pr2[:],
                             start=True, stop=True)
            lg = mlps.tile([32, NT], bf16, tag="lg")
            nc.scalar.activation(lg[:], ps_lg[:], ACTF.Identity, bias=cst["b3vec"][:])

            # ---- softmax over A=32 (transpose to batch-major) ----
            ps_tr = mlpp.tile([128, 128], bf16, tag="mlp")
            for s in range(4):
                nc.tensor.transpose(ps_tr[:, 32 * s:32 * (s + 1)],
                                    lg[:, 128 * s:128 * (s + 1)], cst["ident32"][:])
            esb = outp.tile([128, 128], fp32, tag="e")
            nc.scalar.activation(esb[:], ps_tr[:], ACTF.Exp)
            e3 = esb.rearrange("p (s a) -> p s a", s=4)
            sums = outp.tile([128, 4], fp32, tag="sums")
            nc.vector.tensor_reduce(out=sums[:], in_=e3[:, :, :],
                                    axis=mybir.AxisListType.X, op=ALU.add)
            rec = outp.tile([128, 4], fp32, tag="rec")
            nc.vector.reciprocal(rec[:], sums[:])
            fin = outp.tile([128, 128], fp32, tag="fin")
            fin3 = fin.rearrange("p (s a) -> p s a", s=4)
            rec_b = rec[:].unsqueeze(2).broadcast_to([128, 4, 32])
            nc.vector.tensor_tensor(out=fin3[:, :, :], in0=e3[:, :, :], in1=rec_b,
                                    op=ALU.mult)
            # batch b = 16q + (4*bt + s): partition q stride 16 rows,
            # segment s stride 1 row
            oap = OUT[:]
            oout = bass.AP(
                tensor=oap.tensor, offset=oap.offset + 4 * bt * A,
                ap=[[16 * A, 128], [A, 4], [1, A]],
            )
            nc.sync.dma_start(out=oout, in_=fin3[:, :, :])
    nc.finalize()
    return nc


def kernel(**inputs):
    from concourse.bass_utils import run_bass_kernel_spmd

    if "nc" not in _prog_cache:
        _prog_cache["nc"] = build_program(NCORES)
    nc = _prog_cache["nc"]

    in_maps = make_in_maps(inputs)
    res = run_bass_kernel_spmd(nc, in_maps, list(range(NCORES)))
    out = np.concatenate([res.results[i]["OUT"] for i in range(NCORES)], axis=0)
    return out.astype(np.float32)


# revision 57
# speedup vs baseline: 1.2417x; 1.2417x over previous
"""Trainium2 Bass kernel for nn_DiscretePolicy (gnn_message_passing).

Reference computation:
  Xn = batchnorm(X)  (training-mode, biased var, eps=1e-5)
  ent = Xn[:, 4:].reshape(B, 100, 2)
  me = leaky_relu(ent @ W_me.T + b_me); me_out = mean_k(me)      # [B, 64]
  h = leaky_relu([Xn[:, :4], me_out] @ W1.T + b1)
  h = leaky_relu(h @ W2.T + b2)
  out = softmax(h @ W3.T + b3)

Strategy (8-way batch-parallel, 2048 rows/core):
  - X is pre-expanded on the host to a bf16 column-layout XB [B, 256]
    (block k of 32 cols <- features 28k..28k+31; pad features zero), and
    rotated per-core so chunk 0 is the core's own shard.
  - BatchNorm stats: every core streams the FULL bf16 batch and reduces
    locally (a cross-core AllReduce costs ~135us cold in this harness).
    Sums via PE ones-matmuls on 4 concurrent col-group accumulation
    chains (x-even/x-odd/sq-even/sq-odd); squares on DVE (2x bf16).
  - Own-shard tiles are PE-transposed to feature-major XT [128, 2*2048],
    normalized in place with the batch stats (rsqrt via reciprocal+sqrt).
  - leaky_relu(z) is decomposed as alpha*z + (1-alpha)*relu(z). The linear
    part is folded analytically into the first MLP layer; only
    R = sum_k relu(z_k + b_me) is computed at full resolution:
      * entity matmuls: K=32 zero-padded block weights, one entity-pair per
        matmul, 4 concurrent via tile_position row groups, bf16 PSUM
      * relu+bias split 3 ways across DVE (2x bf16), ACT, and GpSimd
      * pooling over entities: two concurrent PE accumulation chains on
        col-group halves (tile_position=(0,0)/(0,64)), bf16 sel weights
  - MLP in bf16: leaky layers via max identity — h = a*p + (1-a)*relu(p)
    as two accumulating matmuls on (p, relu(p)); softmax via PE transpose
    to batch-major then Exp + reciprocal (logits are O(1): no max-sub).
"""

import sys
import numpy as np

sys.path.insert(0, "/opt/trn_rl_repo")

import ml_dtypes

B_FULL, D, H, A = 16384, 204, 64, 32
NCORES = 8
BL = B_FULL // NCORES          # 2048 rows per core
NBT = 4                        # batch tiles per core
NT = BL // NBT                 # 512 columns per batch tile
K_ENT = 100                    # entities
NPAIR = 50                     # entity pairs (2 entities / matmul)
ALPHA = 0.01                   # jax.nn.leaky_relu default negative_slope
EPS = 1e-5
C = 256                        # padded feature columns (bf16 layout)
DPAD = 228                     # padded feature count (204..227 = 0)

# --- column layout: block k of 32 sbuf columns <- features 28k .. 28k+31 ---
# Pair p (features 4+4p..7+4p) lives in block k=(4+4p)//28 at column
# 4+4p+4k; pairs are 4-aligned and blocks start at multiples of 28 (also
# 4-aligned), so pairs never straddle blocks.
PAIR_COL = np.array([4 + 4 * p + 4 * ((4 + 4 * p) // 28) for p in range(NPAIR)])
for p in range(NPAIR):
    c = PAIR_COL[p]
    k = c // 32
    assert c % 4 == 0 and c % 32 <= 24 and 28 * k <= 4 + 4 * p <= 28 * k + 24


def _feat_of_col():
    f = np.full(C, -1, np.int64)
    for c in range(C):
        k, r = c // 32, c % 32
        if 28 * k + r < D:
            f[c] = 28 * k + r
    return f


FEAT_OF_COL = _feat_of_col()

PAIR_FILL = PAIR_COL // 128            # which transpose block (XT region)
PAIR_PART = PAIR_COL % 128             # partition of first row
PAIR_QUAD = PAIR_PART // 32            # row-group quadrant
PAIR_SLOT = (PAIR_PART % 32) // 4      # slot within quadrant (selects lhsT block)

# round-robin issue order across quadrants
_QLISTS = [[p for p in range(NPAIR) if PAIR_QUAD[p] == g] for g in range(4)]
PAIR_ORDER = []
for t in range(max(len(q) for q in _QLISTS)):
    for g in range(4):
        if t < len(_QLISTS[g]):
            PAIR_ORDER.append(_QLISTS[g][t])
assert len(PAIR_ORDER) == NPAIR

_prog_cache = {}


def _build_host_constants(W_me, b_me, W1, b1, W2, b2, W3, b3):
    bf16 = ml_dtypes.bfloat16
    # Wall [128, 8*128]: for quadrant row r (0..31) and slot m: rows 4m..4m+3
    # hold the entity-pair weight block, other rows zero.  Wall same for all
    # quadrants -> replicate pattern to all 128 partitions.
    pat = np.zeros((32, 8 * 128), np.float32)
    for m in range(8):
        for j in range(2):          # entity within pair
            for e in range(2):      # input dim
                # row 4m+2j+e, columns m*128 + (64j .. 64j+63) = W_me[:, e]
                pat[4 * m + 2 * j + e, m * 128 + 64 * j: m * 128 + 64 * (j + 1)] = W_me[:, e]
        # row 28 (a never-used duplicate column in every quadrant) is set
        # to 1.0 in xt after normalization; putting b_me here folds the
        # bias add into the z matmul, so relu is a single-op max.
        pat[28, m * 128:(m + 1) * 128] = np.tile(b_me, 2)
    Wall = np.tile(pat, (4, 1)).astype(bf16)

    sel = np.zeros((128, 64), np.float32)
    for j in range(2):
        sel[np.arange(64) + 64 * j, np.arange(64)] = 1.0
    selpack = np.concatenate([sel, sel], axis=1).astype(ml_dtypes.float8_e4m3)

    # m-vector masks (feature sums of the pair columns), folded below into
    # the h1 lhsT weights together with the head columns.
    mA2 = np.zeros((128, 2), np.float32)
    mB2 = np.zeros((128, 2), np.float32)
    pair_cols = set()
    for p in range(NPAIR):
        for off in range(4):
            pair_cols.add(int(PAIR_COL[p]) + off)
    for c in range(C):
        f = FEAT_OF_COL[c]
        if c in pair_cols and f >= 4:
            (mA2 if c < 128 else mB2)[c % 128, (f - 4) % 2] = 1.0

    # h1 = (1-a)/K * W1b @ R  +  a/K * (W1b@W_me) @ m_raw  +  W1h @ head:
    # R arrives as two evacuated pool halves (lhsT_R applied to each);
    # the m_raw and head contributions are folded into direct lhsT weights
    # applied to the normalized xt regions.
    W1h = W1[:, :4]
    W1b = W1[:, 4:]
    lhsT_R = (((1.0 - ALPHA) / K_ENT) * W1b.T).astype(bf16).copy()
    coefT = ((ALPHA / K_ENT) * (W1b @ W_me)).T          # [2, 64]
    lhsT_mA = mA2 @ coefT
    lhsT_mA[0:4, :] += W1h.T
    lhsT_mA = lhsT_mA.astype(bf16)
    lhsT_mB = (mB2 @ coefT).astype(bf16)
    b1vec = (b1 + ALPHA * (W1b @ b_me)).reshape(64, 1).astype(np.float32)

    # stacked leaky weights: h_next = lhsT[[a*W],[ (1-a)*W ]] @ [p; relu(p)]
    lhsT_h2 = np.concatenate([(ALPHA * W2).T, ((1.0 - ALPHA) * W2).T],
                             axis=0).astype(bf16)        # [128, 64]
    b2vec = b2.reshape(64, 1).astype(np.float32)
    lhsT_h3 = np.concatenate([(ALPHA * W3).T, ((1.0 - ALPHA) * W3).T],
                             axis=0).astype(bf16)        # [128, 32]
    b3vec = b3.reshape(32, 1).astype(np.float32)

    ident = np.eye(128, dtype=np.float32).astype(bf16)   # logits transpose identity
    ident32 = np.eye(32, dtype=np.float32).astype(bf16)  # logits transpose identity
    onesb = np.ones((128, 32), np.float32).astype(bf16)  # stats ones (M=32: HAM-visible)

    return dict(Wall=Wall, selpack=selpack, lhsT_R=lhsT_R, lhsT_mA=lhsT_mA,
                lhsT_mB=lhsT_mB, b1vec=b1vec, lhsT_h2=lhsT_h2, b2vec=b2vec,
                lhsT_h3=lhsT_h3, b3vec=b3vec, ident=ident, ident32=ident32,
                onesb=onesb)


_COL_MAP = np.array([28 * (c // 32) + c % 32 for c in range(C)])


def _expand_X(X):
    """Host-side: pad, expand to the 256-col window layout, cast bf16."""
    Xp = np.zeros((X.shape[0], DPAD), np.float32)
    Xp[:, :D] = X
    return np.ascontiguousarray(Xp[:, _COL_MAP].astype(ml_dtypes.bfloat16))


def make_in_maps(inputs):
    X = np.asarray(inputs["X"], np.float32)
    consts = _build_host_constants(
        np.asarray(inputs["W_me"], np.float32), np.asarray(inputs["b_me"], np.float32),
        np.asarray(inputs["W1"], np.float32), np.asarray(inputs["b1"], np.float32),
        np.asarray(inputs["W2"], np.float32), np.asarray(inputs["b2"], np.float32),
        np.asarray(inputs["W3"], np.float32), np.asarray(inputs["b3"], np.float32),
    )
    XB = _expand_X(X)
    in_maps = []
    for i in range(NCORES):
        rot = np.ascontiguousarray(np.concatenate([XB[i * BL:], XB[:i * BL]], axis=0))
        m = {"XB": rot}
        m.update(consts)
        in_maps.append(m)
    return in_maps


def build_program(num_devices=NCORES):
    """Emit the SPMD Bass program (identical on every core)."""
    from contextlib import ExitStack
    import concourse.bass as bass
    import concourse.bacc as bacc
    import concourse.tile as tile
    from concourse import mybir

    fp32 = mybir.dt.float32
    bf16 = mybir.dt.bfloat16
    fp8 = mybir.dt.float8e4
    ALU = mybir.AluOpType
    ACTF = mybir.ActivationFunctionType

    nc = bacc.Bacc(None, num_devices=num_devices)

    XB = nc.declare_dram_parameter("XB", [B_FULL, C], bf16, isOutput=False)
    OUT = nc.declare_dram_parameter("OUT", [BL, A], fp32, isOutput=True)
    dparams = {}
    for name, shape, dt in [
        ("Wall", [128, 1024], bf16), ("selpack", [128, 128], fp8),
        ("lhsT_R", [64, 64], bf16), ("lhsT_mA", [128, 64], bf16),
        ("lhsT_mB", [128, 64], bf16),
        ("b1vec", [64, 1], fp32), ("lhsT_h2", [128, 64], bf16),
        ("b2vec", [64, 1], fp32), ("lhsT_h3", [128, 32], bf16),
        ("b3vec", [32, 1], fp32), ("ident", [128, 128], bf16),
        ("ident32", [32, 32], bf16), ("onesb", [128, 32], bf16),
    ]:
        dparams[name] = nc.declare_dram_parameter(name, shape, dt, isOutput=False)

    ntile = BL // 128              # 16 tile-rows per chunk
    nchunk = B_FULL // BL          # 8 chunks (chunk 0 = own shard)

    with tile.TileContext(nc) as tc, ExitStack() as ctx:
        singles = ctx.enter_context(tc.tile_pool(name="singles", bufs=1))
        xtp = ctx.enter_context(tc.tile_pool(name="xtp", bufs=1))
        x0p = ctx.enter_context(tc.tile_pool(name="x0p", bufs=1))
        xstream = ctx.enter_context(tc.tile_pool(name="xstream", bufs=2))

        cst = {}

        def load_const(name):
            p = dparams[name]
            t = singles.tile(list(p.shape), p.dtype, tag=f"cst_{name}")
            nc.sync.dma_start(out=t[:], in_=p[:])
            cst[name] = t

        load_const("onesb")
        load_const("ident")

        # PE heater: the HAM clock gate keeps the PE at 1.2 GHz unless its
        # activity window reads busy.  Dummy wide matmuls (ident x ident)
        # fill idle gaps so the whole kernel runs at 2.4 GHz.
        heatp = ctx.enter_context(tc.tile_pool(name="heatp", bufs=1, space="PSUM"))
        heat = heatp.tile([128, 512], fp32, tag="heat")

        def heater(n=1):
            for _ in range(n):
                nc.tensor.matmul(heat[:, 0:128], cst["ident"][:], cst["ident"][:],
                                 start=True, stop=True)

        def heater512(src):
            # big heater: 512-free dummy matmul reading live SBUF data
            nc.tensor.matmul(heat[:], cst["ident"][:], src, start=True, stop=True)

        heater(30)   # ~3.5us of back-to-back matmuls: un-throttle before stats

        def chunk_dma(ci, pool):
            xc = pool.tile([128, ntile, C], bf16)
            cin = bass.AP(
                tensor=XB[:].tensor, offset=XB[:].offset + ci * BL * C,
                ap=[[ntile * C, 128], [C, ntile], [1, C]],
            )
            nc.sync.dma_start(out=xc[:], in_=cin)
            return xc

        xc0 = chunk_dma(0, x0p)         # own shard: stays resident
        xchunks = {1: chunk_dma(1, xstream)}

        for name in dparams:
            if name not in cst:
                load_const(name)

        # XT: feature-major bf16, region F at cols F*BL .. F*BL+BL
        xt = xtp.tile([128, 2 * BL], bf16)
        xt3 = xt.rearrange("p (s n) -> p s n", s=2)

        # ---------------- pre-phase: load, stats, transpose ----------------
        with ExitStack() as pre:
            sps = pre.enter_context(tc.tile_pool(name="sps", bufs=1, space="PSUM"))
            stp = pre.enter_context(tc.tile_pool(name="stp", bufs=4))
            xsqp = pre.enter_context(tc.tile_pool(name="xsqp", bufs=2))

            # stats: 4 concurrent col-group accumulation chains (x even
            # tiles, x odd tiles, sq even, sq odd) with M=32 outputs so the
            # matmuls register as HAM activity (M=1 sums are invisible to
            # the clock-gate monitor and the PE stays at half clock)
            pstiles = []
            for ch in range(4):
                ps_stc = sps.tile([128, C], fp32, tag=f"ps_st{ch}", name=f"ps_st{ch}")
                pstiles.append(ps_stc)
            onesb = cst["onesb"][:]
            first = [True, True, True, True]

            def stats_mms(xc, ci):
                xq = xsqp.tile([128, ntile, C], bf16)
                nc.vector.tensor_tensor(
                    out=xq.rearrange("p t d -> p (t d)")[:],
                    in0=xc.rearrange("p t d -> p (t d)")[:],
                    in1=xc.rearrange("p t d -> p (t d)")[:], op=ALU.mult)
                last = ci == nchunk - 1
                for i in range(ntile):
                    ch = i % 2
                    nc.tensor.matmul(pstiles[ch][ch * 32:ch * 32 + 32, :], onesb,
                                     xc[:, i, :],
                                     start=first[ch], stop=(last and i >= ntile - 2),
                                     tile_position=(0, 32 * ch))
                    first[ch] = False
                    chq = 2 + ch
                    nc.tensor.matmul(pstiles[chq][chq * 32:chq * 32 + 32, :], onesb,
                                     xq[:, i, :],
                                     start=first[chq], stop=(last and i >= ntile - 2),
                                     tile_position=(0, 32 * chq))
                    first[chq] = False

            stats_mms(xc0, 0)
            # transposes of the own shard (overlap the rest of the stream)
            pps = pre.enter_context(tc.tile_pool(name="pps", bufs=2, space="PSUM"))
            for i in range(ntile):
                pt = pps.tile([128, C], bf16)
                nc.tensor.transpose(pt[:, 0:128], xc0[:, i, 0:128], cst["ident"][:])
                nc.tensor.transpose(pt[:, 128:256], xc0[:, i, 128:256], cst["ident"][:])
                pt3 = pt.rearrange("p (s n) -> p s n", s=2)
                nc.vector.tensor_copy(xt3[:, :, 128 * i:128 * (i + 1)], pt3[:, :, :])
            for ci in range(1, nchunk):
                xc = xchunks.pop(ci) if ci in xchunks else chunk_dma(ci, xstream)
                stats_mms(xc, ci)

            # --- stats: evacuate (fold 4 chains -> 2), reshape, normalize ---
            # (only one PSUM operand allowed per instruction: stage the odd
            # chains through SBUF first)
            st_odd = stp.tile([1, 2 * C], fp32, tag="st_odd")
            nc.scalar.activation(st_odd[:, 0:C], pstiles[1][32:33, :], ACTF.Copy)
            nc.scalar.activation(st_odd[:, C:2 * C], pstiles[3][96:97, :], ACTF.Copy)
            st_sb = stp.tile([1, 2 * C], fp32)
            nc.vector.tensor_tensor(out=st_sb[:, 0:C], in0=pstiles[0][0:1, :],
                                    in1=st_odd[:, 0:C], op=ALU.add)
            nc.vector.tensor_tensor(out=st_sb[:, C:2 * C], in0=pstiles[2][64:65, :],
                                    in1=st_odd[:, C:2 * C], op=ALU.add)
            # poison the stats of the 8 never-used duplicate columns
            # (row 28 of each quadrant, both regions) so normalization
            # writes exactly 1.0 there: with sum=-B*sqrt(M), sumsq=2*B*M,
            # mu=-sqrt(M), var=M -> s=M^-0.5=1e-6, shift=-mu*s=1.0.
            BIGM = 1e12
            for base in (28, 28 + 128):
                sx = bass.AP(tensor=st_sb.tensor, offset=st_sb.offset + base,
                             ap=[[st_sb.ap[0][0], 1], [32, 4]])
                sq = bass.AP(tensor=st_sb.tensor,
                             offset=st_sb.offset + C + base,
                             ap=[[st_sb.ap[0][0], 1], [32, 4]])
                nc.vector.memset(sx, -B_FULL * BIGM ** 0.5)
                nc.vector.memset(sq, 2.0 * B_FULL * BIGM)
            cc_out = nc.dram_tensor("cc_out", [1, 2 * C], fp32)
            nc.sync.dma_start(out=cc_out[:], in_=st_sb[:])
            cc_v = cc_out[:].rearrange("a (e p) -> (a p) e", e=2)
            # per-partition stats for both regions at once: st[p, F] layout
            st = stp.tile([128, 2, 2], fp32, tag="st")   # [p, F, (sx, sq)]
            for F in range(2):
                nc.sync.dma_start(out=st[:, F, :], in_=cc_v[128 * F:128 * (F + 1), :])
            stf = st.rearrange("p f e -> p (f e)")
            muex = stp.tile([128, 4], fp32, tag="muex")  # (muA, ex2A, muB, ex2B)
            nc.vector.tensor_scalar(out=muex[:], in0=stf[:, 0:4], scalar1=1.0 / B_FULL,
                                    scalar2=None, op0=ALU.mult)
            mus = muex[:, 0:4:2]
            ex2 = muex[:, 1:4:2]
            mu2 = stp.tile([128, 2], fp32, tag="mu2")
            nc.vector.tensor_tensor(out=mu2[:], in0=mus, in1=mus, op=ALU.mult)
            vpe = stp.tile([128, 2], fp32, tag="vpe")
            # vpe = (mu2 * -1 + ex2) + eps
            nc.vector.scalar_tensor_tensor(out=vpe[:], in0=mu2[:], scalar=-1.0,
                                           in1=ex2, op0=ALU.mult, op1=ALU.add)
            nc.vector.tensor_scalar(out=vpe[:], in0=vpe[:], scalar1=EPS, scalar2=None,
                                    op0=ALU.add)
            heater(14)   # cover the stats-finalize + normalize PE gap
            rs = stp.tile([128, 2], fp32, tag="rs")
            nc.vector.reciprocal(rs[:], vpe[:])          # 1/(var+eps)
            svec = stp.tile([128, 2], fp32, tag="sv")
            nc.scalar.activation(svec[:], rs[:], ACTF.Sqrt)
            nmvec = stp.tile([128, 2], fp32, tag="nm")
            # nm = (mu * -1) * s
            nc.vector.scalar_tensor_tensor(out=nmvec[:], in0=mus, scalar=-1.0,
                                           in1=svec[:], op0=ALU.mult, op1=ALU.mult)
            for F in range(2):
                nc.vector.tensor_scalar(
                    out=xt[:, BL * F:BL * (F + 1)], in0=xt[:, BL * F:BL * (F + 1)],
                    scalar1=svec[:, F:F + 1], scalar2=nmvec[:, F:F + 1],
                    op0=ALU.mult, op1=ALU.add)

        # ---------------- main phase ----------------
        zpsp = ctx.enter_context(tc.tile_pool(name="zpsp", bufs=1, space="PSUM"))
        accp = ctx.enter_context(tc.tile_pool(name="accp", bufs=1, space="PSUM"))
        mlpp = ctx.enter_context(tc.tile_pool(name="mlpp", bufs=1, space="PSUM"))
        ypool = ctx.enter_context(tc.tile_pool(name="ypool", bufs=4))
        ypoolb = ctx.enter_context(tc.tile_pool(name="ypoolb", bufs=4))
        mlps = ctx.enter_context(tc.tile_pool(name="mlps", bufs=4))
        outp = ctx.enter_context(tc.tile_pool(name="outp", bufs=4))

        zpsA = zpsp.tile([128, 2 * 512], fp32, tag="zpsA")
        zpsB = zpsp.tile([128, 2 * 512], fp32, tag="zpsB")

        groups = [PAIR_ORDER[i:i + 2] for i in range(0, NPAIR, 2)]
        ngrp = len(groups)           # 25 groups of 2 pairs
        DVE_RELU = {0, 2, 4, 7, 9, 11, 13, 16, 18, 20, 22, 24}  # 12 DVE / 13 ACT

        for bt in range(NBT):
            col0 = bt * NT
            # two pool accumulation chains in separate PSUM banks (the
            # col-group halves may not share a bank's accumulation group)
            acc0 = accp.tile([128, NT], fp32, tag="acc0")
            acc1 = accp.tile([128, NT], fp32, tag="acc1")

            def emit_z(gi):
                zps = zpsA if gi % 2 == 0 else zpsB
                for j, p in enumerate(groups[gi]):
                    g = PAIR_QUAD[p]
                    m = PAIR_SLOT[p]
                    F = PAIR_FILL[p]
                    nc.tensor.matmul(
                        zps[:, j * 512:(j + 1) * 512],
                        cst["Wall"][32 * g:32 * (g + 1), 128 * m:128 * (m + 1)],
                        xt[32 * g:32 * (g + 1), BL * F + col0:BL * F + col0 + NT],
                        start=True, stop=True,
                        tile_position=(32 * int(g), 0),
                    )

            ytiles = {}
            selpack3 = cst["selpack"][:].rearrange("p (two f) -> p two f", two=2)

            def emit_relu(gi):
                zps = zpsA if gi % 2 == 0 else zpsB
                y = (ypool if gi % 2 == 0 else ypoolb).tile([128, 2 * 512], fp8)
                ytiles[gi] = y
                zv = zps[:, 0:2 * 512]
                # bias already folded into the z matmul: single-op relu
                # (GpSimd cannot read PSUM, so only DVE and ACT share it)
                if gi in DVE_RELU:
                    nc.vector.tensor_scalar(out=y[:], in0=zv, scalar1=0.0,
                                            scalar2=None, op0=ALU.max)
                else:
                    nc.scalar.activation(y[:], zv, ACTF.Relu)

            # pool chains: chain0 = DoubleRow on col groups 0-1 (DR requires
            # dst partition 0), chain1 = two plain fp8 matmuls on col groups
            # 2-3; ~17/8 group split balances the two chains' cycle counts.
            ch1 = [1, 3, 5, 8, 10, 12, 14, 16, 19, 21, 23]
            ch1set = set(ch1)
            ch0 = [gi for gi in range(ngrp) if gi not in ch1set]

            def emit_pool(gi):
                y = ytiles.pop(gi)
                if gi in ch1set:
                    nc.tensor.matmul(acc1[64:128, :], selpack3[:, 0, :],
                                     y[:, 0:512],
                                     start=(gi == ch1[0]), stop=False,
                                     tile_position=(0, 64))
                    nc.tensor.matmul(acc1[64:128, :], selpack3[:, 0, :],
                                     y[:, 512:1024],
                                     start=False, stop=(gi == ch1[-1]),
                                     tile_position=(0, 64))
                else:
                    y3 = y.rearrange("p (j d) -> p j d", j=2)
                    nc.tensor.matmul(acc0[0:64, :], selpack3, y3[:, :, :],
                                     start=(gi == ch0[0]), stop=(gi == ch0[-1]),
                                     tile_position=(0, 0),
                                     perf_mode=mybir.MatmulPerfMode.DoubleRow)

            # software pipeline: z-matmuls 2 groups ahead of relu (2 PSUM
            # banks; relu(gi-2) must precede z(gi) in program order since
            # they share zps slices), pool lags 2 more.
            for gi in range(ngrp + 4):
                if gi >= 4:
                    emit_pool(gi - 4)
                if 2 <= gi < ngrp + 2:
                    emit_relu(gi - 2)
                if gi < ngrp:
                    emit_z(gi)
                heater512(xt[:, col0:col0 + NT])

            # ---- MLP tail ----
            # evacuate the two pool halves; h1 accumulates their R
            # contributions plus the m_raw/head terms taken directly off xt
            e0 = mlps.tile([64, NT], bf16, tag="e0")
            e1 = mlps.tile([64, NT], bf16, tag="e1")
            nc.vector.tensor_copy(e0[:], acc0[0:64, :])
            nc.scalar.activation(e1[:], acc1[64:128, :], ACTF.Copy)
            ps_h1 = mlpp.tile([64, NT], fp32, tag="mlp")
            nc.tensor.matmul(ps_h1[:], cst["lhsT_R"][:], e0[:],
                             start=True, stop=False)
            nc.tensor.matmul(ps_h1[:], cst["lhsT_R"][:], e1[:],
                             start=False, stop=False)
            nc.tensor.matmul(ps_h1[:], cst["lhsT_mA"][:], xt[:, col0:col0 + NT],
                             start=False, stop=False)
            nc.tensor.matmul(ps_h1[:], cst["lhsT_mB"][:],
                             xt[:, BL + col0:BL + col0 + NT],
                             start=False, stop=True)

            # stacked [p; relu(p)] tiles: one matmul per leaky layer
            pr1 = mlps.tile([128, NT], bf16, tag="pr1")
            nc.vector.tensor_scalar(out=pr1[0:64, :], in0=ps_h1[:],
                                    scalar1=cst["b1vec"][:], scalar2=None,
                                    op0=ALU.add)
            nc.scalar.activation(pr1[64:128, :], ps_h1[:], ACTF.Relu,
                                 bias=cst["b1vec"][:])
            heater()
            ps_h2 = mlpp.tile([64, NT], fp32, tag="mlp")
            nc.tensor.matmul(ps_h2[:], cst["lhsT_h2"][:], pr1[:],
                             start=True, stop=True)
            pr2 = mlps.tile([128, NT], bf16, tag="pr2")
            nc.vector.tensor_scalar(out=pr2[0:64, :], in0=ps_h2[:],
                                    scalar1=cst["b2vec"][:], scalar2=None,
                                    op0=ALU.add)
            nc.scalar.activation(pr2[64:128, :], ps_h2[:], ACTF.Relu,
                                 bias=cst["b2vec"][:])
            heater()
            ps_lg = mlpp.tile([32, NT], fp32, tag="mlp")
            nc.tensor.matmul(ps_lg[:], cst["lhsT_h3"][:], pr2[:],
                             start=True, stop=True)
            lg = mlps.tile([32, NT], bf16, tag="lg")
            nc.scalar.activation(lg[:], ps_lg[:], ACTF.Identity, bias=cst["b3vec"][:])

            # ---- softmax over A=32 (transpose to batch-major) ----
            ps_tr = mlpp.tile([128, 128], bf16, tag="mlp")
            for s in range(4):
                nc.tensor.transpose(ps_tr[:, 32 * s:32 * (s + 1)],
                                    lg[:, 128 * s:128 * (s + 1)], cst["ident32"][:])
            esb = outp.tile([128, 128], fp32, tag="e")
            nc.scalar.activation(esb[:], ps_tr[:], ACTF.Exp)
            e3 = esb.rearrange("p (s a) -> p s a", s=4)
            sums = outp.tile([128, 4], fp32, tag="sums")
            nc.vector.tensor_reduce(out=sums[:], in_=e3[:, :, :],
                                    axis=mybir.AxisListType.X, op=ALU.add)
            rec = outp.tile([128, 4], fp32, tag="rec")
            nc.vector.reciprocal(rec[:], sums[:])
            fin = outp.tile([128, 128], fp32, tag="fin")
            fin3 = fin.rearrange("p (s a) -> p s a", s=4)
            rec_b = rec[:].unsqueeze(2).broadcast_to([128, 4, 32])
            nc.vector.tensor_tensor(out=fin3[:, :, :], in0=e3[:, :, :], in1=rec_b,
                                    op=ALU.mult)
            # batch b = 16q + (4*bt + s): partition q stride 16 rows,
            # segment s stride 1 row
            oap = OUT[:]
            oout = bass.AP(
                tensor=oap.tensor, offset=oap.offset + 4 * bt * A,
                ap=[[16 * A, 128], [A, 4], [1, A]],
            )
            nc.sync.dma_start(out=oout, in_=fin3[:, :, :])
    nc.finalize()
    return nc


def kernel(**inputs):
    from concourse.bass_utils import run_bass_kernel_spmd

    if "nc" not in _prog_cache:
        _prog_cache["nc"] = build_program(NCORES)
    nc = _prog_cache["nc"]

    in_maps = make_in_maps(inputs)
    res = run_bass_kernel_spmd(nc, in_maps, list(range(NCORES)))
    out = np.concatenate([res.results[i]["OUT"] for i in range(NCORES)], axis=0)
    return out.astype(np.float32)


# revision 64
# speedup vs baseline: 1.2749x; 1.0267x over previous
"""Trainium2 Bass kernel for nn_DiscretePolicy (gnn_message_passing).

Reference computation:
  Xn = batchnorm(X)  (training-mode, biased var, eps=1e-5)
  ent = Xn[:, 4:].reshape(B, 100, 2)
  me = leaky_relu(ent @ W_me.T + b_me); me_out = mean_k(me)      # [B, 64]
  h = leaky_relu([Xn[:, :4], me_out] @ W1.T + b1)
  h = leaky_relu(h @ W2.T + b2)
  out = softmax(h @ W3.T + b3)

Strategy (8-way batch-parallel, 2048 rows/core):
  - X is pre-expanded on the host to a bf16 column-layout XB [B, 256]
    (block k of 32 cols <- features 28k..28k+31; pad features zero), and
    rotated per-core so chunk 0 is the core's own shard.
  - BatchNorm stats: every core streams the FULL bf16 batch and reduces
    locally (a cross-core AllReduce costs ~135us cold in this harness).
    Sums via PE ones-matmuls on 4 concurrent col-group accumulation
    chains (x-even/x-odd/sq-even/sq-odd); squares on DVE (2x bf16).
  - Own-shard tiles are PE-transposed to feature-major XT [128, 2*2048],
    normalized in place with the batch stats (rsqrt via reciprocal+sqrt).
  - leaky_relu(z) is decomposed as alpha*z + (1-alpha)*relu(z). The linear
    part is folded analytically into the first MLP layer; only
    R = sum_k relu(z_k + b_me) is computed at full resolution:
      * entity matmuls: K=32 zero-padded block weights, one entity-pair per
        matmul, 4 concurrent via tile_position row groups, bf16 PSUM
      * relu+bias split 3 ways across DVE (2x bf16), ACT, and GpSimd
      * pooling over entities: two concurrent PE accumulation chains on
        col-group halves (tile_position=(0,0)/(0,64)), bf16 sel weights
  - MLP in bf16: leaky layers via max identity — h = a*p + (1-a)*relu(p)
    as two accumulating matmuls on (p, relu(p)); softmax via PE transpose
    to batch-major then Exp + reciprocal (logits are O(1): no max-sub).
"""

import sys
import numpy as np

sys.path.insert(0, "/opt/trn_rl_repo")

import ml_dtypes

B_FULL, D, H, A = 16384, 204, 64, 32
NCORES = 8
BL = B_FULL // NCORES          # 2048 rows per core
NBT = 4                        # batch tiles per core
NT = BL // NBT                 # 512 columns per batch tile
K_ENT = 100                    # entities
NPAIR = 50                     # entity pairs (2 entities / matmul)
ALPHA = 0.01                   # jax.nn.leaky_relu default negative_slope
EPS = 1e-5
C = 256                        # padded feature columns (bf16 layout)
DPAD = 228                     # padded feature count (204..227 = 0)

# --- column layout: block k of 32 sbuf columns <- features 28k .. 28k+31 ---
# Pair p (features 4+4p..7+4p) lives in block k=(4+4p)//28 at column
# 4+4p+4k; pairs are 4-aligned and blocks start at multiples of 28 (also
# 4-aligned), so pairs never straddle blocks.
PAIR_COL = np.array([4 + 4 * p + 4 * ((4 + 4 * p) // 28) for p in range(NPAIR)])
for p in range(NPAIR):
    c = PAIR_COL[p]
    k = c // 32
    assert c % 4 == 0 and c % 32 <= 24 and 28 * k <= 4 + 4 * p <= 28 * k + 24


def _feat_of_col():
    f = np.full(C, -1, np.int64)
    for c in range(C):
        k, r = c // 32, c % 32
        if 28 * k + r < D:
            f[c] = 28 * k + r
    return f


FEAT_OF_COL = _feat_of_col()

PAIR_FILL = PAIR_COL // 128            # which transpose block (XT region)
PAIR_PART = PAIR_COL % 128             # partition of first row
PAIR_QUAD = PAIR_PART // 32            # row-group quadrant
PAIR_SLOT = (PAIR_PART % 32) // 4      # slot within quadrant (selects lhsT block)

# round-robin issue order across quadrants
_QLISTS = [[p for p in range(NPAIR) if PAIR_QUAD[p] == g] for g in range(4)]
PAIR_ORDER = []
for t in range(max(len(q) for q in _QLISTS)):
    for g in range(4):
        if t < len(_QLISTS[g]):
            PAIR_ORDER.append(_QLISTS[g][t])
assert len(PAIR_ORDER) == NPAIR

_prog_cache = {}


def _build_host_constants(W_me, b_me, W1, b1, W2, b2, W3, b3):
    bf16 = ml_dtypes.bfloat16
    # Wall [128, 8*128]: for quadrant row r (0..31) and slot m: rows 4m..4m+3
    # hold the entity-pair weight block, other rows zero.  Wall same for all
    # quadrants -> replicate pattern to all 128 partitions.
    pat = np.zeros((32, 8 * 128), np.float32)
    for m in range(8):
        for j in range(2):          # entity within pair
            for e in range(2):      # input dim
                # row 4m+2j+e, columns m*128 + (64j .. 64j+63) = W_me[:, e]
                pat[4 * m + 2 * j + e, m * 128 + 64 * j: m * 128 + 64 * (j + 1)] = W_me[:, e]
        # row 28 (a never-used duplicate column in every quadrant) is set
        # to 1.0 in xt after normalization; putting b_me here folds the
        # bias add into the z matmul, so relu is a single-op max.
        pat[28, m * 128:(m + 1) * 128] = np.tile(b_me, 2)
    Wall = np.tile(pat, (4, 1)).astype(bf16)

    sel = np.zeros((128, 64), np.float32)
    for j in range(2):
        sel[np.arange(64) + 64 * j, np.arange(64)] = 1.0
    selpack = np.concatenate([sel, sel], axis=1).astype(ml_dtypes.float8_e4m3)

    # m-vector masks (feature sums of the pair columns), folded below into
    # the h1 lhsT weights together with the head columns.
    mA2 = np.zeros((128, 2), np.float32)
    mB2 = np.zeros((128, 2), np.float32)
    pair_cols = set()
    for p in range(NPAIR):
        for off in range(4):
            pair_cols.add(int(PAIR_COL[p]) + off)
    for c in range(C):
        f = FEAT_OF_COL[c]
        if c in pair_cols and f >= 4:
            (mA2 if c < 128 else mB2)[c % 128, (f - 4) % 2] = 1.0

    # h1 = (1-a)/K * W1b @ R  +  a/K * (W1b@W_me) @ m_raw  +  W1h @ head:
    # R arrives as two evacuated pool halves (lhsT_R applied to each);
    # the m_raw and head contributions are folded into direct lhsT weights
    # applied to the normalized xt regions.
    W1h = W1[:, :4]
    W1b = W1[:, 4:]
    lhsT_R = (((1.0 - ALPHA) / K_ENT) * W1b.T).astype(bf16).copy()
    coefT = ((ALPHA / K_ENT) * (W1b @ W_me)).T          # [2, 64]
    lhsT_mA = mA2 @ coefT
    lhsT_mA[0:4, :] += W1h.T
    lhsT_mA = lhsT_mA.astype(bf16)
    lhsT_mB = (mB2 @ coefT).astype(bf16)
    b1vec = (b1 + ALPHA * (W1b @ b_me)).reshape(64, 1).astype(np.float32)

    # stacked leaky weights: h_next = lhsT[[a*W],[ (1-a)*W ]] @ [p; relu(p)]
    lhsT_h2 = np.concatenate([(ALPHA * W2).T, ((1.0 - ALPHA) * W2).T],
                             axis=0).astype(bf16)        # [128, 64]
    b2vec = b2.reshape(64, 1).astype(np.float32)
    lhsT_h3 = np.concatenate([(ALPHA * W3).T, ((1.0 - ALPHA) * W3).T],
                             axis=0).astype(bf16)        # [128, 32]
    b3vec = b3.reshape(32, 1).astype(np.float32)

    ident = np.eye(128, dtype=np.float32).astype(bf16)   # logits transpose identity
    ident32 = np.eye(32, dtype=np.float32).astype(bf16)  # logits transpose identity
    onesb = np.ones((128, 32), np.float32).astype(bf16)  # stats ones (M=32: HAM-visible)

    return dict(Wall=Wall, selpack=selpack, lhsT_R=lhsT_R, lhsT_mA=lhsT_mA,
                lhsT_mB=lhsT_mB, b1vec=b1vec, lhsT_h2=lhsT_h2, b2vec=b2vec,
                lhsT_h3=lhsT_h3, b3vec=b3vec, ident=ident, ident32=ident32,
                onesb=onesb)


_COL_MAP = np.array([28 * (c // 32) + c % 32 for c in range(C)])


def _expand_X(X):
    """Host-side: pad, expand to the 256-col window layout, cast bf16."""
    Xp = np.zeros((X.shape[0], DPAD), np.float32)
    Xp[:, :D] = X
    return np.ascontiguousarray(Xp[:, _COL_MAP].astype(ml_dtypes.bfloat16))


def make_in_maps(inputs):
    X = np.asarray(inputs["X"], np.float32)
    consts = _build_host_constants(
        np.asarray(inputs["W_me"], np.float32), np.asarray(inputs["b_me"], np.float32),
        np.asarray(inputs["W1"], np.float32), np.asarray(inputs["b1"], np.float32),
        np.asarray(inputs["W2"], np.float32), np.asarray(inputs["b2"], np.float32),
        np.asarray(inputs["W3"], np.float32), np.asarray(inputs["b3"], np.float32),
    )
    XB = _expand_X(X)
    in_maps = []
    for i in range(NCORES):
        rot = np.ascontiguousarray(np.concatenate([XB[i * BL:], XB[:i * BL]], axis=0))
        m = {"XB": rot}
        m.update(consts)
        in_maps.append(m)
    return in_maps


def build_program(num_devices=NCORES):
    """Emit the SPMD Bass program (identical on every core)."""
    from contextlib import ExitStack
    import concourse.bass as bass
    import concourse.bacc as bacc
    import concourse.tile as tile
    from concourse import mybir

    fp32 = mybir.dt.float32
    bf16 = mybir.dt.bfloat16
    fp8 = mybir.dt.float8e4
    ALU = mybir.AluOpType
    ACTF = mybir.ActivationFunctionType

    nc = bacc.Bacc(None, num_devices=num_devices)

    XB = nc.declare_dram_parameter("XB", [B_FULL, C], bf16, isOutput=False)
    OUT = nc.declare_dram_parameter("OUT", [BL, A], fp32, isOutput=True)
    dparams = {}
    for name, shape, dt in [
        ("Wall", [128, 1024], bf16), ("selpack", [128, 128], fp8),
        ("lhsT_R", [64, 64], bf16), ("lhsT_mA", [128, 64], bf16),
        ("lhsT_mB", [128, 64], bf16),
        ("b1vec", [64, 1], fp32), ("lhsT_h2", [128, 64], bf16),
        ("b2vec", [64, 1], fp32), ("lhsT_h3", [128, 32], bf16),
        ("b3vec", [32, 1], fp32), ("ident", [128, 128], bf16),
        ("ident32", [32, 32], bf16), ("onesb", [128, 32], bf16),
    ]:
        dparams[name] = nc.declare_dram_parameter(name, shape, dt, isOutput=False)

    ntile = BL // 128              # 16 tile-rows per chunk
    nchunk = B_FULL // BL          # 8 chunks (chunk 0 = own shard)

    with tile.TileContext(nc) as tc, ExitStack() as ctx:
        singles = ctx.enter_context(tc.tile_pool(name="singles", bufs=1))
        xtp = ctx.enter_context(tc.tile_pool(name="xtp", bufs=1))
        x0p = ctx.enter_context(tc.tile_pool(name="x0p", bufs=1))
        xstream = ctx.enter_context(tc.tile_pool(name="xstream", bufs=3))

        cst = {}

        def load_const(name):
            p = dparams[name]
            t = singles.tile(list(p.shape), p.dtype, tag=f"cst_{name}")
            nc.sync.dma_start(out=t[:], in_=p[:])
            cst[name] = t

        load_const("onesb")
        load_const("ident")

        # PE heater: the HAM clock gate keeps the PE at 1.2 GHz unless its
        # activity window reads busy.  Dummy wide matmuls (ident x ident)
        # fill idle gaps so the whole kernel runs at 2.4 GHz.
        heatp = ctx.enter_context(tc.tile_pool(name="heatp", bufs=1, space="PSUM"))
        heat = heatp.tile([128, 512], fp32, tag="heat")

        def heater(n=1):
            for _ in range(n):
                nc.tensor.matmul(heat[:, 0:128], cst["ident"][:], cst["ident"][:],
                                 start=True, stop=True)

        def heater512(src):
            # big heater: 512-free dummy matmul reading live SBUF data
            nc.tensor.matmul(heat[:], cst["ident"][:], src, start=True, stop=True)

        heater(30)   # ~3.5us of back-to-back matmuls: un-throttle before stats

        def chunk_dma(ci, pool):
            xc = pool.tile([128, ntile, C], bf16)
            cin = bass.AP(
                tensor=XB[:].tensor, offset=XB[:].offset + ci * BL * C,
                ap=[[ntile * C, 128], [C, ntile], [1, C]],
            )
            nc.sync.dma_start(out=xc[:], in_=cin)
            return xc

        xc0 = chunk_dma(0, x0p)         # own shard: stays resident
        xchunks = {1: chunk_dma(1, xstream), 2: chunk_dma(2, xstream)}

        for name in dparams:
            if name not in cst:
                load_const(name)

        # XT: feature-major bf16, region F at cols F*BL .. F*BL+BL
        xt = xtp.tile([128, 2 * BL], bf16)
        xt3 = xt.rearrange("p (s n) -> p s n", s=2)

        # ---------------- pre-phase: load, stats, transpose ----------------
        with ExitStack() as pre:
            sps = pre.enter_context(tc.tile_pool(name="sps", bufs=1, space="PSUM"))
            stp = pre.enter_context(tc.tile_pool(name="stp", bufs=4))
            xsqp = pre.enter_context(tc.tile_pool(name="xsqp", bufs=2))

            # stats: 4 concurrent col-group accumulation chains (x even
            # tiles, x odd tiles, sq even, sq odd) with M=32 outputs so the
            # matmuls register as HAM activity (M=1 sums are invisible to
            # the clock-gate monitor and the PE stays at half clock)
            pstiles = []
            for ch in range(4):
                ps_stc = sps.tile([128, C], fp32, tag=f"ps_st{ch}", name=f"ps_st{ch}")
                pstiles.append(ps_stc)
            onesb = cst["onesb"][:]
            first = [True, True, True, True]

            pending_xq = [None]

            def emit_xq(xq, last):
                for i in range(ntile):
                    chq = 2 + i % 2
                    nc.tensor.matmul(pstiles[chq][chq * 32:chq * 32 + 32, :], onesb,
                                     xq[:, i, :],
                                     start=first[chq], stop=(last and i >= ntile - 2),
                                     tile_position=(0, 32 * chq))
                    first[chq] = False

            def stats_mms(xc, ci):
                # square on DVE; its sum-matmuls are emitted one chunk late
                # so the PE never stalls waiting for the square
                xq = xsqp.tile([128, ntile, C], bf16)
                nc.vector.tensor_tensor(
                    out=xq.rearrange("p t d -> p (t d)")[:],
                    in0=xc.rearrange("p t d -> p (t d)")[:],
                    in1=xc.rearrange("p t d -> p (t d)")[:], op=ALU.mult)
                last = ci == nchunk - 1
                for i in range(ntile):
                    ch = i % 2
                    nc.tensor.matmul(pstiles[ch][ch * 32:ch * 32 + 32, :], onesb,
                                     xc[:, i, :],
                                     start=first[ch], stop=(last and i >= ntile - 2),
                                     tile_position=(0, 32 * ch))
                    first[ch] = False
                if pending_xq[0] is not None:
                    emit_xq(*pending_xq[0])
                pending_xq[0] = (xq, last)

            stats_mms(xc0, 0)
            # transposes of the own shard (overlap the rest of the stream)
            pps = pre.enter_context(tc.tile_pool(name="pps", bufs=2, space="PSUM"))
            for i in range(ntile):
                pt = pps.tile([128, C], bf16)
                nc.tensor.transpose(pt[:, 0:128], xc0[:, i, 0:128], cst["ident"][:])
                nc.tensor.transpose(pt[:, 128:256], xc0[:, i, 128:256], cst["ident"][:])
                pt3 = pt.rearrange("p (s n) -> p s n", s=2)
                nc.vector.tensor_copy(xt3[:, :, 128 * i:128 * (i + 1)], pt3[:, :, :])
            for ci in range(1, nchunk):
                xc = xchunks.pop(ci) if ci in xchunks else chunk_dma(ci, xstream)
                stats_mms(xc, ci)
            emit_xq(*pending_xq[0])   # flush the lagged last-chunk squares

            # --- stats: evacuate (fold 4 chains -> 2), reshape, normalize ---
            # (only one PSUM operand allowed per instruction: stage the odd
            # chains through SBUF first)
            st_odd = stp.tile([1, 2 * C], fp32, tag="st_odd")
            nc.scalar.activation(st_odd[:, 0:C], pstiles[1][32:33, :], ACTF.Copy)
            nc.scalar.activation(st_odd[:, C:2 * C], pstiles[3][96:97, :], ACTF.Copy)
            st_sb = stp.tile([1, 2 * C], fp32)
            nc.vector.tensor_tensor(out=st_sb[:, 0:C], in0=pstiles[0][0:1, :],
                                    in1=st_odd[:, 0:C], op=ALU.add)
            nc.vector.tensor_tensor(out=st_sb[:, C:2 * C], in0=pstiles[2][64:65, :],
                                    in1=st_odd[:, C:2 * C], op=ALU.add)
            # poison the stats of the 8 never-used duplicate columns
            # (row 28 of each quadrant, both regions) so normalization
            # writes exactly 1.0 there: with sum=-B*sqrt(M), sumsq=2*B*M,
            # mu=-sqrt(M), var=M -> s=M^-0.5=1e-6, shift=-mu*s=1.0.
            BIGM = 1e12
            for base in (28, 28 + 128):
                sx = bass.AP(tensor=st_sb.tensor, offset=st_sb.offset + base,
                             ap=[[st_sb.ap[0][0], 1], [32, 4]])
                sq = bass.AP(tensor=st_sb.tensor,
                             offset=st_sb.offset + C + base,
                             ap=[[st_sb.ap[0][0], 1], [32, 4]])
                nc.vector.memset(sx, -B_FULL * BIGM ** 0.5)
                nc.vector.memset(sq, 2.0 * B_FULL * BIGM)
            cc_out = nc.dram_tensor("cc_out", [1, 2 * C], fp32)
            nc.sync.dma_start(out=cc_out[:], in_=st_sb[:])
            cc_v = cc_out[:].rearrange("a (e p) -> (a p) e", e=2)
            # per-partition stats for both regions at once: st[p, F] layout
            st = stp.tile([128, 2, 2], fp32, tag="st")   # [p, F, (sx, sq)]
            for F in range(2):
                nc.sync.dma_start(out=st[:, F, :], in_=cc_v[128 * F:128 * (F + 1), :])
            stf = st.rearrange("p f e -> p (f e)")
            muex = stp.tile([128, 4], fp32, tag="muex")  # (muA, ex2A, muB, ex2B)
            nc.vector.tensor_scalar(out=muex[:], in0=stf[:, 0:4], scalar1=1.0 / B_FULL,
                                    scalar2=None, op0=ALU.mult)
            mus = muex[:, 0:4:2]
            ex2 = muex[:, 1:4:2]
            mu2 = stp.tile([128, 2], fp32, tag="mu2")
            nc.vector.tensor_tensor(out=mu2[:], in0=mus, in1=mus, op=ALU.mult)
            vpe = stp.tile([128, 2], fp32, tag="vpe")
            # vpe = (mu2 * -1 + ex2) + eps
            nc.vector.scalar_tensor_tensor(out=vpe[:], in0=mu2[:], scalar=-1.0,
                                           in1=ex2, op0=ALU.mult, op1=ALU.add)
            nc.vector.tensor_scalar(out=vpe[:], in0=vpe[:], scalar1=EPS, scalar2=None,
                                    op0=ALU.add)
            heater(14)   # cover the stats-finalize + normalize PE gap
            rs = stp.tile([128, 2], fp32, tag="rs")
            nc.vector.reciprocal(rs[:], vpe[:])          # 1/(var+eps)
            svec = stp.tile([128, 2], fp32, tag="sv")
            nc.scalar.activation(svec[:], rs[:], ACTF.Sqrt)
            nmvec = stp.tile([128, 2], fp32, tag="nm")
            # nm = (mu * -1) * s
            nc.vector.scalar_tensor_tensor(out=nmvec[:], in0=mus, scalar=-1.0,
                                           in1=svec[:], op0=ALU.mult, op1=ALU.mult)
            for F in range(2):
                nc.vector.tensor_scalar(
                    out=xt[:, BL * F:BL * (F + 1)], in0=xt[:, BL * F:BL * (F + 1)],
                    scalar1=svec[:, F:F + 1], scalar2=nmvec[:, F:F + 1],
                    op0=ALU.mult, op1=ALU.add)

        # ---------------- main phase ----------------
        zpsp = ctx.enter_context(tc.tile_pool(name="zpsp", bufs=1, space="PSUM"))
        accp = ctx.enter_context(tc.tile_pool(name="accp", bufs=1, space="PSUM"))
        mlpp = ctx.enter_context(tc.tile_pool(name="mlpp", bufs=1, space="PSUM"))
        ypool = ctx.enter_context(tc.tile_pool(name="ypool", bufs=4))
        ypoolb = ctx.enter_context(tc.tile_pool(name="ypoolb", bufs=4))
        mlps = ctx.enter_context(tc.tile_pool(name="mlps", bufs=4))
        outp = ctx.enter_context(tc.tile_pool(name="outp", bufs=4))

        zpsA = zpsp.tile([128, 2 * 512], fp32, tag="zpsA")
        zpsB = zpsp.tile([128, 2 * 512], fp32, tag="zpsB")

        groups = [PAIR_ORDER[i:i + 2] for i in range(0, NPAIR, 2)]
        ngrp = len(groups)           # 25 groups of 2 pairs
        DVE_RELU = {0, 2, 4, 7, 9, 11, 13, 16, 18, 20, 22, 24}  # 12 DVE / 13 ACT
        pending_softmax = [None]

        for bt in range(NBT):
            col0 = bt * NT
            # two pool accumulation chains in separate PSUM banks (the
            # col-group halves may not share a bank's accumulation group)
            acc0 = accp.tile([128, NT], fp32, tag="acc0")
            acc1 = accp.tile([128, NT], fp32, tag="acc1")

            def emit_z(gi):
                zps = zpsA if gi % 2 == 0 else zpsB
                for j, p in enumerate(groups[gi]):
                    g = PAIR_QUAD[p]
                    m = PAIR_SLOT[p]
                    F = PAIR_FILL[p]
                    nc.tensor.matmul(
                        zps[:, j * 512:(j + 1) * 512],
                        cst["Wall"][32 * g:32 * (g + 1), 128 * m:128 * (m + 1)],
                        xt[32 * g:32 * (g + 1), BL * F + col0:BL * F + col0 + NT],
                        start=True, stop=True,
                        tile_position=(32 * int(g), 0),
                    )

            ytiles = {}
            selpack3 = cst["selpack"][:].rearrange("p (two f) -> p two f", two=2)

            def emit_relu(gi):
                zps = zpsA if gi % 2 == 0 else zpsB
                y = (ypool if gi % 2 == 0 else ypoolb).tile([128, 2 * 512], fp8)
                ytiles[gi] = y
                zv = zps[:, 0:2 * 512]
                # bias already folded into the z matmul: single-op relu
                # (GpSimd cannot read PSUM, so only DVE and ACT share it)
                if gi in DVE_RELU:
                    nc.vector.tensor_scalar(out=y[:], in0=zv, scalar1=0.0,
                                            scalar2=None, op0=ALU.max)
                else:
                    nc.scalar.activation(y[:], zv, ACTF.Relu)

            # pool chains: chain0 = DoubleRow on col groups 0-1 (DR requires
            # dst partition 0), chain1 = two plain fp8 matmuls on col groups
            # 2-3; ~17/8 group split balances the two chains' cycle counts.
            ch1 = [1, 3, 5, 8, 10, 12, 14, 16, 19, 21, 23]
            ch1set = set(ch1)
            ch0 = [gi for gi in range(ngrp) if gi not in ch1set]

            def emit_pool(gi):
                y = ytiles.pop(gi)
                if gi in ch1set:
                    nc.tensor.matmul(acc1[64:128, :], selpack3[:, 0, :],
                                     y[:, 0:512],
                                     start=(gi == ch1[0]), stop=False,
                                     tile_position=(0, 64))
                    nc.tensor.matmul(acc1[64:128, :], selpack3[:, 0, :],
                                     y[:, 512:1024],
                                     start=False, stop=(gi == ch1[-1]),
                                     tile_position=(0, 64))
                else:
                    y3 = y.rearrange("p (j d) -> p j d", j=2)
                    nc.tensor.matmul(acc0[0:64, :], selpack3, y3[:, :, :],
                                     start=(gi == ch0[0]), stop=(gi == ch0[-1]),
                                     tile_position=(0, 0),
                                     perf_mode=mybir.MatmulPerfMode.DoubleRow)

            # software pipeline: z-matmuls 2 groups ahead of relu (2 PSUM
            # banks; relu(gi-2) must precede z(gi) in program order since
            # they share zps slices), pool lags 2 more.
            for gi in range(ngrp + 4):
                if gi >= 4:
                    emit_pool(gi - 4)
                if 2 <= gi < ngrp + 2:
                    emit_relu(gi - 2)
                if gi < ngrp:
                    emit_z(gi)
                heater512(xt[:, col0:col0 + NT])
                if gi == 2 and pending_softmax[0] is not None:
                    # previous bt's softmax: by now its logits are long done,
                    # so the transposes don't bubble the PE queue
                    pending_softmax[0]()
                    pending_softmax[0] = None

            # ---- MLP tail ----
            # evacuate the two pool halves; h1 accumulates their R
            # contributions plus the m_raw/head terms taken directly off xt
            e0 = mlps.tile([64, NT], bf16, tag="e0")
            e1 = mlps.tile([64, NT], bf16, tag="e1")
            nc.vector.tensor_copy(e0[:], acc0[0:64, :])
            nc.scalar.activation(e1[:], acc1[64:128, :], ACTF.Copy)
            ps_h1 = mlpp.tile([64, NT], fp32, tag="mlp")
            nc.tensor.matmul(ps_h1[:], cst["lhsT_R"][:], e0[:],
                             start=True, stop=False)
            nc.tensor.matmul(ps_h1[:], cst["lhsT_R"][:], e1[:],
                             start=False, stop=False)
            nc.tensor.matmul(ps_h1[:], cst["lhsT_mA"][:], xt[:, col0:col0 + NT],
                             start=False, stop=False)
            nc.tensor.matmul(ps_h1[:], cst["lhsT_mB"][:],
                             xt[:, BL + col0:BL + col0 + NT],
                             start=False, stop=True)

            # stacked [p; relu(p)] tiles: one matmul per leaky layer
            pr1 = mlps.tile([128, NT], bf16, tag="pr1")
            nc.vector.tensor_scalar(out=pr1[0:64, :], in0=ps_h1[:],
                                    scalar1=cst["b1vec"][:], scalar2=None,
                                    op0=ALU.add)
            nc.scalar.activation(pr1[64:128, :], ps_h1[:], ACTF.Relu,
                                 bias=cst["b1vec"][:])
            heater()
            ps_h2 = mlpp.tile([64, NT], fp32, tag="mlp")
            nc.tensor.matmul(ps_h2[:], cst["lhsT_h2"][:], pr1[:],
                             start=True, stop=True)
            pr2 = mlps.tile([128, NT], bf16, tag="pr2")
            nc.vector.tensor_scalar(out=pr2[0:64, :], in0=ps_h2[:],
                                    scalar1=cst["b2vec"][:], scalar2=None,
                                    op0=ALU.add)
            nc.scalar.activation(pr2[64:128, :], ps_h2[:], ACTF.Relu,
                                 bias=cst["b2vec"][:])
            heater()
            ps_lg = mlpp.tile([32, NT], fp32, tag="mlp")
            nc.tensor.matmul(ps_lg[:], cst["lhsT_h3"][:], pr2[:],
                             start=True, stop=True)
            lg = mlps.tile([32, NT], bf16, tag="lg")
            nc.scalar.activation(lg[:], ps_lg[:], ACTF.Identity, bias=cst["b3vec"][:])

            # ---- softmax over A=32 (transpose to batch-major) ----
            # emitted during the next bt's pipeline (or at the end)
            def softmax_block(bt=bt, lg=lg):
                ps_tr = mlpp.tile([128, 128], bf16, tag="mlp")
                for s in range(4):
                    nc.tensor.transpose(ps_tr[:, 32 * s:32 * (s + 1)],
                                        lg[:, 128 * s:128 * (s + 1)],
                                        cst["ident32"][:])
                esb = outp.tile([128, 128], fp32, tag="e")
                nc.scalar.activation(esb[:], ps_tr[:], ACTF.Exp)
                e3 = esb.rearrange("p (s a) -> p s a", s=4)
                sums = outp.tile([128, 4], fp32, tag="sums")
                nc.vector.tensor_reduce(out=sums[:], in_=e3[:, :, :],
                                        axis=mybir.AxisListType.X, op=ALU.add)
                rec = outp.tile([128, 4], fp32, tag="rec")
                nc.vector.reciprocal(rec[:], sums[:])
                fin = outp.tile([128, 128], fp32, tag="fin")
                fin3 = fin.rearrange("p (s a) -> p s a", s=4)
                rec_b = rec[:].unsqueeze(2).broadcast_to([128, 4, 32])
                nc.vector.tensor_tensor(out=fin3[:, :, :], in0=e3[:, :, :],
                                        in1=rec_b, op=ALU.mult)
                # batch b = 16q + (4*bt + s): partition q stride 16 rows,
                # segment s stride 1 row
                oap = OUT[:]
                oout = bass.AP(
                    tensor=oap.tensor, offset=oap.offset + 4 * bt * A,
                    ap=[[16 * A, 128], [A, 4], [1, A]],
                )
                nc.sync.dma_start(out=oout, in_=fin3[:, :, :])

            pending_softmax[0] = softmax_block
        pending_softmax[0]()
    nc.finalize()
    return nc


def kernel(**inputs):
    from concourse.bass_utils import run_bass_kernel_spmd

    if "nc" not in _prog_cache:
        _prog_cache["nc"] = build_program(NCORES)
    nc = _prog_cache["nc"]

    in_maps = make_in_maps(inputs)
    res = run_bass_kernel_spmd(nc, in_maps, list(range(NCORES)))
    out = np.concatenate([res.results[i]["OUT"] for i in range(NCORES)], axis=0)
    return out.astype(np.float32)
